# revision 28
# baseline (speedup 1.0000x reference)
"""Trainium2 Bass kernel for an autoregressive LSTM (warmup scan + decode).

Math (Keras LSTMCell, gate order i,f,g,o in the reference):
    z = x @ Wk + h @ Wr + b
    c = sigmoid(f)*c + sigmoid(i)*tanh(g)
    h = sigmoid(o)*tanh(c)
Warmup over T=256 input steps, then S=64 autoregressive decode steps through
a dense head p = h @ Wd + bd fed back as the next input.

Sharding: pure data-parallel over batch, 1024/8 = 128 examples per core
(128 = SBUF partition count). Weights replicated. No collectives.

Per-core layout: z is computed as [batch=128 part, 4096 gates] with the
batch-transposed activations as the matmul stationary operand and the
weights streaming, N=512 per PSUM bank. Gate columns are pre-permuted on the
host into NW=4 1024-wide "waves" [i_q|f_q|o_q|g_q] over unit-quarters; each
wave is a 2-bank PSUM tile (pool bufs=3) whose gate math starts while later
waves are still in the matmul stream. Within a wave the matmuls run k-outer
(x first, then h chunks) so the next step's PE work never waits on the
previous step's late h chunks. h is transposed back to [units, batch]
chunk-major layout with ONE merged DMA xbar transpose per wave (~1.25us
fixed cost regardless of size), off the compute engines.

fp8: the recurrent h @ Wr matmul — 94% of the MACs — runs in fp8-e4m3 with
perf_mode=DoubleRow (2 contraction rows per PE cell: K=256 per stationary
load, ~1.7x PE throughput) for all warmup steps except the last
WARM_BF16_TAIL and for all decode steps. Errors injected by early-step
quantization decay through the forget-gate contraction, so only the tail
steps' precision reaches the output (validated numerically: rel err is
within noise of all-bf16). All weights are pre-scaled by SC=64 so the fp8
weights sit in e4m3's normal range; the gate activations undo it with
scale=1/SC. h itself lies in (-1,1) where e4m3 needs no scaling. The fp8
transposed state is produced by a per-wave gpsimd copy of the bf16 hT tile
(the DMA xbar transpose only moves 2-byte elements).
"""

import sys

sys.path.insert(0, "/opt/trn_rl_repo")

import numpy as np

import concourse.bass as bass
import concourse.bacc as bacc
import concourse.mybir as mybir
from concourse.tile import TileContext
from concourse.bass_utils import run_bass_kernel_spmd

F32 = mybir.dt.float32
BF16 = mybir.dt.bfloat16
FP8 = mybir.dt.float8e4
NPBF16 = mybir.dt.np(mybir.dt.bfloat16)
NPFP8 = mybir.dt.np(mybir.dt.float8e4)
AF = mybir.ActivationFunctionType
DR = mybir.MatmulPerfMode.DoubleRow

B, T, I, U, S = 1024, 256, 64, 1024, 64
NCORES = 8
BC = B // NCORES          # 128 batch per core
KX = I + 1                # x rows + ones row for folded bias
NU = U // 128             # 8 recurrent k-chunks (bf16)
NCH = NU // 2             # 4 DoubleRow k-chunks (fp8, K=256 each)
XBLK = 4                  # warmup steps per input-stream DMA block

NW = 4                    # waves per step (each covers U/NW units, 4U/NW z-cols)
QW = U // NW              # units per wave
WW = 4 * QW               # z columns per wave
NB = WW // 512            # PSUM banks (512-col matmuls) per wave

SC = 64.0                 # global weight scale: fp8 Wr lands in e4m3 normal range
SCI = 1.0 / SC

WARM_BF16_TAIL = 12       # last warmup steps in bf16 (error decay buffer)
DECODE_FP8 = False


def _gate_perm():
    """Column permutation: reference gate order [i|f|g|o] (1024 each) ->
    NW waves of [i_q | f_q | o_q | g_q] (QW each)."""
    i0, f0, g0, o0 = 0, U, 2 * U, 3 * U
    parts = []
    for w in range(NW):
        for g in (i0, f0, o0, g0):
            parts.append(np.arange(QW) + g + w * QW)
    return np.concatenate(parts)


def build_nc(n_warm=T, n_dec=S - 1):
    nc = bacc.Bacc()

    n_steps = n_warm + n_dec

    def is_fp8(i):
        if i < n_warm:
            return i < n_warm - WARM_BF16_TAIL
        return DECODE_FP8

    nblk = (n_warm + XBLK - 1) // XBLK
    xTbD = nc.declare_dram_parameter("xTb", [nblk, KX, XBLK * BC], BF16, isOutput=False)
    WkD = nc.declare_dram_parameter("Wk", [KX, 4 * U], BF16, isOutput=False)
    WrD = nc.declare_dram_parameter("Wr", [128, NU, 4 * U], BF16, isOutput=False)
    Wr8D = nc.declare_dram_parameter("Wr8", [128, NCH, 2, 4 * U], FP8, isOutput=False)
    WdD = nc.declare_dram_parameter("Wd", [128, NU, I], BF16, isOutput=False)
    bdD = nc.declare_dram_parameter("bdc", [I, 1], F32, isOutput=False)
    outD = nc.declare_dram_parameter("out", [n_dec + 1, I, BC], F32, isOutput=True)

    with TileContext(nc) as tc:
        with (
            tc.tile_pool(name="const", bufs=1) as cpool,
            tc.tile_pool(name="xp", bufs=2) as xpool,
            tc.tile_pool(name="state", bufs=3) as hpool,
            tc.tile_pool(name="state8", bufs=3) as hpool8,
            tc.tile_pool(name="gates", bufs=2) as gpool,
            tc.tile_pool(name="psum", bufs=2, space="PSUM") as zpool,
        ):
            Wk_sb = cpool.tile([KX, 4 * U], BF16)
            Wr_sb = cpool.tile([128, NU, 4 * U], BF16)
            Wr8_sb = cpool.tile([128, NCH, 2, 4 * U], FP8)
            Wd_sb = cpool.tile([128, NU, I], BF16)
            bd_sb = cpool.tile([I, 1], F32)
            c_sb = cpool.tile([128, U], F32)
            nc.sync.dma_start(Wk_sb[:], WkD[:])
            nc.sync.dma_start(Wr_sb[:], WrD[:])
            nc.sync.dma_start(Wr8_sb[:], Wr8D[:])
            nc.sync.dma_start(Wd_sb[:], WdD[:])
            nc.sync.dma_start(bd_sb[:], bdD[:])
            nc.gpsimd.memset(c_sb[:], 0.0)

            nch = QW // 128   # 2 transposed 128-blocks per wave (= 1 DR chunk)

            def gates_a(z, w):
                """Front half of wave w's gate math: ACTs + c update.

                Returns the tiles needed by gates_b. Split so the tanh(c) of
                wave w (which waits on the DVE c-chain) can be emitted AFTER
                the next wave's sigmoid in the strict-FIFO ScalarE queue —
                head-of-line blocking there was the v2 cycle bottleneck.
                """
                sig = gpool.tile([128, 3 * QW], F32, tag="sig", name="sig")
                tg = gpool.tile([128, QW], F32, tag="tg", name="tg")
                # one sigmoid over the contiguous [i|f|o] block: 3x fewer
                # ScalarE fixed costs (ScalarE is near-critical at fp8 rate)
                nc.scalar.activation(sig[:], z[:, 0 : 3 * QW], AF.Sigmoid, scale=SCI)
                nc.scalar.activation(tg[:], z[:, 3 * QW :], AF.Tanh, scale=SCI)
                cw = c_sb[:, w * QW : (w + 1) * QW]
                t1 = gpool.tile([128, QW], F32, tag="t1", name="t1")
                t2 = gpool.tile([128, QW], F32, tag="t2", name="t2")
                nc.vector.tensor_mul(t1[:], sig[:, QW : 2 * QW], cw)
                nc.vector.tensor_mul(t2[:], sig[:, 0:QW], tg[:])
                nc.vector.tensor_add(cw, t1[:], t2[:])
                return sig

            pair_state = {}

            def gates_b(sig, w, hT_new, hT8_new):
                """Back half: tanh(c), h, transpose.

                bf16 mode (hT_new): h -> bf16 tile -> per-wave xbar transpose.
                fp8 mode (hT8_new): h is written as fp8 directly by the DVE
                mul, byte-interleaved with the partner wave of its pair
                (w0,w1)/(w2,w3); ONE 2-byte xbar transpose per pair then
                yields the DoubleRow stationary layout in place — no cast,
                no gpsimd, half the transposes. (Verified on HW: DR accepts
                lhsT APs with Ko stride 256B / M stride 2B.)
                """
                cw = c_sb[:, w * QW : (w + 1) * QW]
                tcc = gpool.tile([128, QW], F32, tag="tcc", name="tcc")
                nc.scalar.activation(tcc[:], cw, AF.Tanh)
                assert hT8_new is None, "fp8 output is produced by gates_pair"
                if hT_new is not None:
                    hbf = gpool.tile([128, QW], BF16, tag="hbf", name="hbf")
                    nc.vector.tensor_mul(hbf[:], sig[:, 2 * QW : 3 * QW], tcc[:])
                    # One merged xbar transpose per wave: DMAT cost is ~1.25us
                    # nearly independent of size, so [128, QW] -> [128, nch, 128]
                    # in a single instruction beats per-chunk transposes.
                    nc.sync.dma_start_transpose(
                        hT_new[:, w * nch : (w + 1) * nch, :], hbf[:]
                    )

            def emit_gates(z, w, hT_new, hT8_new, merged=False):
                """Unsplit gate math (bf16 steps): f-first split ACT ordering."""
                sig = gpool.tile([128, 3 * QW], F32, tag="sig", name="sig")
                tg = gpool.tile([128, QW], F32, tag="tg", name="tg")
                if merged:
                    nc.scalar.activation(sig[:], z[:, 0 : 3 * QW], AF.Sigmoid, scale=SCI)
                    nc.scalar.activation(tg[:], z[:, 3 * QW :], AF.Tanh, scale=SCI)
                else:
                    nc.scalar.activation(
                        sig[:, QW : 2 * QW], z[:, QW : 2 * QW], AF.Sigmoid, scale=SCI
                    )
                    nc.scalar.activation(tg[:], z[:, 3 * QW :], AF.Tanh, scale=SCI)
                    nc.scalar.activation(sig[:, 0:QW], z[:, 0:QW], AF.Sigmoid, scale=SCI)
                    nc.scalar.activation(
                        sig[:, 2 * QW : 3 * QW], z[:, 2 * QW : 3 * QW], AF.Sigmoid,
                        scale=SCI,
                    )
                cw = c_sb[:, w * QW : (w + 1) * QW]
                t1 = gpool.tile([128, QW], F32, tag="t1", name="t1")
                t2 = gpool.tile([128, QW], F32, tag="t2", name="t2")
                nc.vector.tensor_mul(t1[:], sig[:, QW : 2 * QW], cw)
                nc.vector.tensor_mul(t2[:], sig[:, 0:QW], tg[:])
                nc.vector.tensor_add(cw, t1[:], t2[:])
                gates_b(sig, w, hT_new, hT8_new)

            def h8chunk(hT8, ch):
                """DoubleRow lhsT AP [128, 2, 128] for chunk ch (units of
                wave ch) out of the pair-transposed container."""
                v = hT8[:, ch // 2, :, :].bitcast(FP8).rearrange(
                    "p a (m s) -> p a m s", s=2
                )
                return v[:, :, :, ch % 2]

            def gates_pair(zP, p, hT_new, hT8_new):
                """Gate math for wave pair p (waves 2p, 2p+1) in ONE set of
                ACT/DVE ops over [128, 2, *] strided APs: halves the ScalarE
                fixed-cost (12 -> 6 ACTs/step), which was delaying the z-PSUM
                reads that gate the next step's x-matmuls."""
                zv = zP[:].rearrange("q (j c) -> q j c", j=2)
                sig = gpool.tile([128, 2, 3 * QW], F32, tag="sigP", name="sigP")
                tg = gpool.tile([128, 2, QW], F32, tag="tgP", name="tgP")
                nc.scalar.activation(sig[:], zv[:, :, 0 : 3 * QW], AF.Sigmoid, scale=SCI)
                nc.scalar.activation(tg[:], zv[:, :, 3 * QW :], AF.Tanh, scale=SCI)
                c3 = c_sb[:, p * 2 * QW : (p + 1) * 2 * QW].rearrange(
                    "q (j c) -> q j c", j=2
                )
                t1 = gpool.tile([128, 2, QW], F32, tag="t1P", name="t1P")
                t2 = gpool.tile([128, 2, QW], F32, tag="t2P", name="t2P")
                nc.vector.tensor_mul(t1[:], sig[:, :, QW : 2 * QW], c3)
                nc.vector.tensor_mul(t2[:], sig[:, :, 0:QW], tg[:])
                nc.vector.tensor_add(c3, t1[:], t2[:])
                tcc = gpool.tile([128, 2, QW], F32, tag="tccP", name="tccP")
                nc.scalar.activation(tcc[:], c3, AF.Tanh)
                if hT8_new is not None:
                    hp8 = gpool.tile([128, QW, 2], FP8, tag="hp8", name="hp8")
                    nc.vector.tensor_mul(
                        hp8[:].rearrange("q m s -> q s m"),
                        sig[:, :, 2 * QW : 3 * QW],
                        tcc[:],
                    )
                    nc.sync.dma_start_transpose(
                        hT8_new[:, p, :, :], hp8[:].bitcast(BF16)
                    )
                if hT_new is not None:
                    hbf = gpool.tile([128, 2, QW], BF16, tag="hbfP", name="hbfP")
                    nc.vector.tensor_mul(hbf[:], sig[:, :, 2 * QW : 3 * QW], tcc[:])
                    for j in range(2):
                        nc.sync.dma_start_transpose(
                            hT_new[:, (2 * p + j) * nch : (2 * p + j + 1) * nch, :],
                            hbf[:, j, :],
                        )

            def emit_step_fp8(x_lhsT, hT8_prev, fp8_out):
                """One fp8 LSTM step (DoubleRow h matmuls).

                PE order: 8 x-matmuls as an h-free prefix; then w0's ch0/ch1,
                w1's full section (closes first), w0's postponed ch2/ch3,
                w2, w3. Closes stay spread for the ScalarE pipeline while
                each chunk's first consumption lands at/after the previous
                step's produce tail (gate chain + pair transpose). Gate
                emission is software-pipelined one wave deep.
                """
                hT_new = (
                    None
                    if fp8_out
                    else hpool.tile([128, NU, 128], BF16, tag="hT", name="hT_new")
                )
                hT8_new = (
                    hpool8.tile([128, 2, 2, 128], BF16, tag="hT8", name="hT8_new")
                    if fp8_out
                    else None
                )
                zP = [
                    zpool.tile([128, 2 * WW], F32, tag="z", name=f"zP{p}")
                    for p in range(2)
                ]
                zs = [zP[w // 2][:, (w % 2) * WW : (w % 2 + 1) * WW] for w in range(NW)]
                x_only = hT8_prev is None
                # x-prefix: pair-0's waves first (their z frees first)
                for w in (1, 0, 2, 3):
                    for n in range(NB):
                        nc.tensor.matmul(
                            zs[w][:, n * 512 : (n + 1) * 512],
                            x_lhsT,
                            Wk_sb[:, WW * w + n * 512 : WW * w + (n + 1) * 512],
                            start=True,
                            stop=x_only,
                        )

                if x_only:
                    gates_pair(zP[0], 0, hT_new, hT8_new)
                    gates_pair(zP[1], 1, hT_new, hT8_new)
                    return hT_new, hT8_new

                def dr(w, ch, stop=False):
                    for n in range(NB):
                        nc.tensor.matmul(
                            zs[w][:, n * 512 : (n + 1) * 512],
                            h8chunk(hT8_prev, ch),
                            Wr8_sb[:, ch, :, WW * w + n * 512 : WW * w + (n + 1) * 512],
                            start=False,
                            stop=stop,
                            perf_mode=DR,
                        )

                dr(0, 0)
                dr(0, 1)
                for ch in range(NCH):          # w1 closes first
                    dr(1, ch, stop=(ch == NCH - 1))
                dr(0, 2)
                dr(0, 3, stop=True)            # w0's postponed chunks close pair 0
                # pair-0's full gate chain: its transpose feeds ch0/ch1,
                # the first chunks the next step consumes.
                gates_pair(zP[0], 0, hT_new, hT8_new)
                for ch in range(NCH):
                    dr(2, ch, stop=(ch == NCH - 1))
                for ch in range(NCH):
                    dr(3, ch, stop=(ch == NCH - 1))
                gates_pair(zP[1], 1, hT_new, hT8_new)
                return hT_new, hT8_new

            def emit_step(x_lhsT, hT_prev, x_first):
                """One bf16 LSTM step; returns (hT_new bf16, None)."""
                hT_new = hpool.tile([128, NU, 128], BF16, tag="hT", name="hT_new")
                hT8_new = None
                zpair = None
                for w in range(NW):
                    base = WW * w
                    if w % 2 == 0:
                        zpair = zpool.tile([128, 2 * WW], F32, tag="z", name="z")
                    z = zpair[:, (w % 2) * WW : (w % 2 + 1) * WW]
                    ks = []
                    if x_first:
                        ks.append(("x", 0))
                    if hT_prev is not None:
                        ks += [("h", u) for u in range(NU)]
                    if not x_first:
                        ks.append(("x", 0))
                    # k-outer / n-inner: the first-emitted matmuls depend on
                    # operands ready earliest (x, then low h chunks), so the
                    # PE can start the next step while the previous step's
                    # late h chunks are still in flight through the
                    # gate-math chain.
                    for ki, (kind, kv) in enumerate(ks):
                        st, sp = ki == 0, ki == len(ks) - 1
                        for n in range(NB):
                            zsl = z[:, n * 512 : (n + 1) * 512]
                            cs = slice(base + n * 512, base + (n + 1) * 512)
                            if kind == "x":
                                nc.tensor.matmul(
                                    zsl, x_lhsT, Wk_sb[:, cs], start=st, stop=sp
                                )
                            else:
                                nc.tensor.matmul(
                                    zsl,
                                    hT_prev[:, kv, :],
                                    Wr_sb[:, kv, cs],
                                    start=st,
                                    stop=sp,
                                )
                    emit_gates(z, w, hT_new, hT8_new, merged=False)
                return hT_new, hT8_new

            def emit_dense(hT_cur, out_idx, feedback, zp=None):
                if zp is None:
                    zp = zpool.tile([128, 2 * WW], F32, tag="z", name="zdense")
                pp = zp[0:I, 0:BC]
                for u in range(NU):
                    nc.tensor.matmul(
                        pp,
                        Wd_sb[:, u, :],
                        hT_cur[:, u, :],
                        start=(u == 0),
                        stop=(u == NU - 1),
                    )
                if feedback:
                    pt = gpool.tile([KX, BC], BF16, tag="pT", name="pT")
                    nc.gpsimd.memset(pt[I : I + 1, :], 1.0)
                    nc.scalar.activation(pt[0:I, :], pp, AF.Identity, bias=bd_sb[:])
                else:
                    pt = None
                pf = gpool.tile([I, BC], F32, tag="pf", name="pf")
                nc.scalar.activation(pf[:], pp, AF.Identity, bias=bd_sb[:])
                nc.scalar.dma_start(outD[out_idx], pf[:])
                return pt

            def emit_step_dec(hT_prev, out_idx):
                """Decode step: consumes hT_prev for BOTH the recurrent
                matmuls and the dense head of the PREVIOUS step's output
                (out_idx), whose result pt feeds this step's x-part. The
                dense matmuls run mid-stream (after wave 1's h-section,
                when hT_prev's late chunks have landed) and all x-matmuls
                come after pt, so the PE never idles waiting for the
                h-transpose tail + dense chain between steps."""
                hT_new = hpool.tile([128, NU, 128], BF16, tag="hT", name="hT_new")
                zp = zpool.tile([128, 2 * WW], F32, tag="z", name="zdense")
                zP = [
                    zpool.tile([128, 2 * WW], F32, tag="z", name=f"zP{p}")
                    for p in range(2)
                ]
                zs = [zP[w // 2][:, (w % 2) * WW : (w % 2 + 1) * WW] for w in range(NW)]

                def hsec(w):
                    for u in range(NU):
                        for n in range(NB):
                            nc.tensor.matmul(
                                zs[w][:, n * 512 : (n + 1) * 512],
                                hT_prev[:, u, :],
                                Wr_sb[:, u, WW * w + n * 512 : WW * w + (n + 1) * 512],
                                start=(u == 0),
                                stop=False,
                            )

                def xsec(w):
                    for n in range(NB):
                        nc.tensor.matmul(
                            zs[w][:, n * 512 : (n + 1) * 512],
                            pt[:],
                            Wk_sb[:, WW * w + n * 512 : WW * w + (n + 1) * 512],
                            start=False,
                            stop=True,
                        )

                hsec(0)
                hsec(1)
                pt = emit_dense(hT_prev, out_idx, feedback=True, zp=zp)
                xsec(0)
                emit_gates(zs[0], 0, hT_new, None)
                xsec(1)
                emit_gates(zs[1], 1, hT_new, None)
                for w in (2, 3):
                    hsec(w)
                    xsec(w)
                    emit_gates(zs[w], w, hT_new, None)
                return hT_new

            hT = hT8 = None
            # prefetch input blocks one block (XBLK steps) ahead so the
            # stream DMA never sits on the first x-matmul's critical path
            nblk_used = (n_warm + XBLK - 1) // XBLK
            xtiles = {}
            if nblk_used > 0:
                xtiles[0] = xpool.tile([KX, XBLK * BC], BF16, tag="xblk", name="xblk")
                nc.sync.dma_start(xtiles[0][:], xTbD[0])
            for t in range(n_warm):
                b = t // XBLK
                s = t % XBLK
                f_out = is_fp8(t + 1) if t + 1 < n_steps else False
                xl = xtiles[b][:, s * BC : (s + 1) * BC]
                if is_fp8(t):
                    hT, hT8 = emit_step_fp8(xl, hT8, f_out)
                else:
                    hT, hT8 = emit_step(xl, hT, x_first=True)
                if t % XBLK == 0 and b + 1 < nblk_used:
                    # prefetch the next input block; emitted after the step so
                    # it queues behind this step's critical hT transposes
                    xtiles[b + 1] = xpool.tile([KX, XBLK * BC], BF16, tag="xblk", name="xblk")
                    nc.sync.dma_start(xtiles[b + 1][:], xTbD[b + 1])
                xtiles.pop(b - 1, None)
            if n_dec == 0:
                emit_dense(hT, 0, feedback=False)
            else:
                for s_ in range(1, n_dec + 1):
                    hT = emit_step_dec(hT, s_ - 1)
                emit_dense(hT, n_dec, feedback=False)

    nc.finalize()
    return nc


def prep_in_maps(inputs, Wk, Wr, b, Wd, bd, n_warm=T):
    """Host-side sharding + layout. inputs [B, T, I] fp32; returns 8 in_maps."""
    perm = _gate_perm()
    Wk_aug = np.concatenate(
        [np.asarray(Wk, np.float32), np.asarray(b, np.float32)[None, :]], axis=0
    )
    Wk_p = (Wk_aug[:, perm] * SC).astype(NPBF16)               # [65, 4096] scaled
    Wr_s = np.asarray(Wr, np.float32)[:, perm] * SC            # [1024, 4096] scaled
    Wr_p = Wr_s.reshape(NU, 128, 4 * U).transpose(1, 0, 2).astype(NPBF16).copy()
    # DoubleRow layout: [p, ch, j, n] = Wr_s[ch*256 + j*128 + p, n]
    Wr_8 = (
        Wr_s.reshape(NCH, 2, 128, 4 * U).transpose(2, 0, 1, 3).astype(NPFP8).copy()
    )
    Wd_p = np.asarray(Wd, np.float32).reshape(NU, 128, I).transpose(1, 0, 2).astype(NPBF16).copy()
    bd_c = np.asarray(bd, np.float32).reshape(I, 1).copy()

    x = np.asarray(inputs, np.float32)
    nblk = (n_warm + XBLK - 1) // XBLK
    in_maps = []
    for c in range(NCORES):
        xc = x[c * BC : (c + 1) * BC, :n_warm]                 # [BC, n_warm, I]
        xT = np.transpose(xc, (1, 2, 0))                       # [n_warm, I, BC]
        xTa = np.concatenate([xT, np.ones((n_warm, 1, BC), np.float32)], axis=1)
        if nblk * XBLK != n_warm:
            pad = np.zeros((nblk * XBLK - n_warm, KX, BC), np.float32)
            xTa = np.concatenate([xTa, pad], axis=0)
        xTb = (
            xTa.reshape(nblk, XBLK, KX, BC)
            .transpose(0, 2, 1, 3)
            .reshape(nblk, KX, XBLK * BC)
            .astype(NPBF16)
            .copy()
        )
        in_maps.append(
            {"xTb": xTb, "Wk": Wk_p, "Wr": Wr_p, "Wr8": Wr_8, "Wd": Wd_p, "bdc": bd_c}
        )
    return in_maps


_NC_CACHE = {}


def _get_nc(n_warm, n_dec):
    key = (n_warm, n_dec)
    if key not in _NC_CACHE:
        _NC_CACHE[key] = build_nc(n_warm, n_dec)
    return _NC_CACHE[key]


def run(inputs, Wk, Wr, b, Wd, bd, n_warm, n_dec, trace=False, **kw):
    nc = _get_nc(n_warm, n_dec)
    in_maps = prep_in_maps(inputs, Wk, Wr, b, Wd, bd, n_warm)
    res = run_bass_kernel_spmd(nc, in_maps, list(range(NCORES)), trace=trace, **kw)
    outs = [np.asarray(res.results[c]["out"], np.float32) for c in range(NCORES)]
    # out[c]: [n_dec+1, I, BC] -> preds [B, n_dec+1, I]
    preds = np.concatenate([o.transpose(2, 0, 1) for o in outs], axis=0)
    return preds, res


def kernel(inputs, Wk, Wr, b, Wd, bd, output_indices, output_steps):
    n_dec = int(output_steps) - 1
    preds, _ = run(inputs, Wk, Wr, b, Wd, bd, T, n_dec)
    idx = np.asarray(output_indices, np.int64)
    return np.take(preds, idx, axis=-1).astype(np.float32)


# revision 36
# speedup vs baseline: 1.2924x; 1.2924x over previous
"""Trainium2 Bass kernel for an autoregressive LSTM (warmup scan + decode).

Math (Keras LSTMCell, gate order i,f,g,o in the reference):
    z = x @ Wk + h @ Wr + b
    c = sigmoid(f)*c + sigmoid(i)*tanh(g)
    h = sigmoid(o)*tanh(c)
Warmup over T=256 input steps, then S=64 autoregressive decode steps through
a dense head p = h @ Wd + bd fed back as the next input.

Sharding: pure data-parallel over batch, 1024/8 = 128 examples per core
(128 = SBUF partition count). Weights replicated. No collectives.

Per-core layout: z is computed as [batch=128 part, 4096 gates] with the
batch-transposed activations as the matmul stationary operand and the
weights streaming, N=512 per PSUM bank. Gate columns are pre-permuted on the
host into NW=4 1024-wide "waves" [i_q|f_q|o_q|g_q] over unit-quarters; each
wave is a 2-bank PSUM tile (pool bufs=3) whose gate math starts while later
waves are still in the matmul stream. Within a wave the matmuls run k-outer
(x first, then h chunks) so the next step's PE work never waits on the
previous step's late h chunks. h is transposed back to [units, batch]
chunk-major layout with ONE merged DMA xbar transpose per wave (~1.25us
fixed cost regardless of size), off the compute engines.

fp8: the recurrent h @ Wr matmul — 94% of the MACs — runs in fp8-e4m3 with
perf_mode=DoubleRow (2 contraction rows per PE cell: K=256 per stationary
load, ~1.7x PE throughput) for all warmup steps except the last
WARM_BF16_TAIL and for all decode steps. Errors injected by early-step
quantization decay through the forget-gate contraction, so only the tail
steps' precision reaches the output (validated numerically: rel err is
within noise of all-bf16). All weights are pre-scaled by SC=64 so the fp8
weights sit in e4m3's normal range; the gate activations undo it with
scale=1/SC. h itself lies in (-1,1) where e4m3 needs no scaling. The fp8
transposed state is produced by a per-wave gpsimd copy of the bf16 hT tile
(the DMA xbar transpose only moves 2-byte elements).
"""

import sys

sys.path.insert(0, "/opt/trn_rl_repo")

import numpy as np

import concourse.bass as bass
import concourse.bacc as bacc
import concourse.mybir as mybir
from concourse.tile import TileContext
from concourse.bass_utils import run_bass_kernel_spmd

F32 = mybir.dt.float32
BF16 = mybir.dt.bfloat16
FP8 = mybir.dt.float8e4
NPBF16 = mybir.dt.np(mybir.dt.bfloat16)
NPFP8 = mybir.dt.np(mybir.dt.float8e4)
AF = mybir.ActivationFunctionType
DR = mybir.MatmulPerfMode.DoubleRow

B, T, I, U, S = 1024, 256, 64, 1024, 64
NCORES = 8
BC = B // NCORES          # 128 batch per core
KX = I + 1                # x rows + ones row for folded bias
NU = U // 128             # 8 recurrent k-chunks (bf16)
NCH = NU // 2             # 4 DoubleRow k-chunks (fp8, K=256 each)
XBLK = 4                  # warmup steps per input-stream DMA block

NW = 4                    # waves per step (each covers U/NW units, 4U/NW z-cols)
QW = U // NW              # units per wave
WW = 4 * QW               # z columns per wave
NB = WW // 512            # PSUM banks (512-col matmuls) per wave

SC = 64.0                 # global weight scale: fp8 Wr lands in e4m3 normal range
SCI = 1.0 / SC

WARM_BF16_TAIL = 12       # last warmup steps in bf16 (error decay buffer)
DECODE_FP8 = False


def _gate_perm():
    """Column permutation: reference gate order [i|f|g|o] (1024 each) ->
    NW waves of [i_q | f_q | o_q | g_q] (QW each)."""
    i0, f0, g0, o0 = 0, U, 2 * U, 3 * U
    parts = []
    for w in range(NW):
        for g in (i0, f0, o0, g0):
            parts.append(np.arange(QW) + g + w * QW)
    return np.concatenate(parts)


def build_nc(n_warm=T, n_dec=S - 1):
    nc = bacc.Bacc()

    n_steps = n_warm + n_dec

    def is_fp8(i):
        if i < n_warm:
            return i < n_warm - WARM_BF16_TAIL
        return DECODE_FP8

    nblk = (n_warm + XBLK - 1) // XBLK
    xTbD = nc.declare_dram_parameter("xTb", [nblk, KX, XBLK * BC], BF16, isOutput=False)
    WkD = nc.declare_dram_parameter("Wk", [KX, 4 * U], BF16, isOutput=False)
    WrD = nc.declare_dram_parameter("Wr", [128, NU, 4 * U], BF16, isOutput=False)
    Wr8D = nc.declare_dram_parameter("Wr8", [128, NCH, 2, 4 * U], FP8, isOutput=False)
    WdD = nc.declare_dram_parameter("Wd", [128, NU, I], BF16, isOutput=False)
    bdD = nc.declare_dram_parameter("bdc", [I, 1], F32, isOutput=False)
    outD = nc.declare_dram_parameter("out", [n_dec + 1, I, BC], F32, isOutput=True)

    with TileContext(nc) as tc:
        with (
            tc.tile_pool(name="const", bufs=1) as cpool,
            tc.tile_pool(name="xp", bufs=2) as xpool,
            tc.tile_pool(name="state", bufs=3) as hpool,
            tc.tile_pool(name="state8", bufs=3) as hpool8,
            tc.tile_pool(name="gates", bufs=2) as gpool,
            tc.tile_pool(name="psum", bufs=4, space="PSUM") as zpool,
        ):
            Wk_sb = cpool.tile([KX, 4 * U], BF16)
            Wr_sb = cpool.tile([128, NU, 4 * U], BF16)
            Wr8_sb = cpool.tile([128, NCH, 2, 4 * U], FP8)
            Wd_sb = cpool.tile([128, NU, I], BF16)
            bd_sb = cpool.tile([I, 1], F32)
            c_sb = cpool.tile([128, U], F32)
            nc.sync.dma_start(Wk_sb[:], WkD[:])
            nc.sync.dma_start(Wr_sb[:], WrD[:])
            nc.sync.dma_start(Wr8_sb[:], Wr8D[:])
            nc.sync.dma_start(Wd_sb[:], WdD[:])
            nc.sync.dma_start(bd_sb[:], bdD[:])
            nc.gpsimd.memset(c_sb[:], 0.0)

            nch = QW // 128   # 2 transposed 128-blocks per wave (= 1 DR chunk)

            def gates_a(z, w):
                """Front half of wave w's gate math: ACTs + c update.

                Returns the tiles needed by gates_b. Split so the tanh(c) of
                wave w (which waits on the DVE c-chain) can be emitted AFTER
                the next wave's sigmoid in the strict-FIFO ScalarE queue —
                head-of-line blocking there was the v2 cycle bottleneck.
                """
                sig = gpool.tile([128, 3 * QW], F32, tag="sig", name="sig")
                tg = gpool.tile([128, QW], F32, tag="tg", name="tg")
                # one sigmoid over the contiguous [i|f|o] block: 3x fewer
                # ScalarE fixed costs (ScalarE is near-critical at fp8 rate)
                nc.scalar.activation(sig[:], z[:, 0 : 3 * QW], AF.Sigmoid, scale=SCI)
                nc.scalar.activation(tg[:], z[:, 3 * QW :], AF.Tanh, scale=SCI)
                cw = c_sb[:, w * QW : (w + 1) * QW]
                t1 = gpool.tile([128, QW], F32, tag="t1", name="t1")
                t2 = gpool.tile([128, QW], F32, tag="t2", name="t2")
                nc.vector.tensor_mul(t1[:], sig[:, QW : 2 * QW], cw)
                nc.vector.tensor_mul(t2[:], sig[:, 0:QW], tg[:])
                nc.vector.tensor_add(cw, t1[:], t2[:])
                return sig

            pair_state = {}

            def gates_b(sig, w, hT_new, hT8_new):
                """Back half: tanh(c), h, transpose.

                bf16 mode (hT_new): h -> bf16 tile -> per-wave xbar transpose.
                fp8 mode (hT8_new): h is written as fp8 directly by the DVE
                mul, byte-interleaved with the partner wave of its pair
                (w0,w1)/(w2,w3); ONE 2-byte xbar transpose per pair then
                yields the DoubleRow stationary layout in place — no cast,
                no gpsimd, half the transposes. (Verified on HW: DR accepts
                lhsT APs with Ko stride 256B / M stride 2B.)
                """
                cw = c_sb[:, w * QW : (w + 1) * QW]
                tcc = gpool.tile([128, QW], F32, tag="tcc", name="tcc")
                nc.scalar.activation(tcc[:], cw, AF.Tanh)
                if hT8_new is not None:
                    k, s = PAIRK[w], SLOT[w]
                    if s == 0:
                        pair_state[k] = gpool.tile(
                            [128, QW, 2], FP8, tag="hp8", name="hp8"
                        )
                    hp8 = pair_state[k]
                    nc.vector.tensor_mul(
                        hp8[:, :, s], sig[:, 2 * QW : 3 * QW], tcc[:]
                    )
                    if s == 1:
                        nc.sync.dma_start_transpose(
                            hT8_new[:, k, :, :], hp8[:].bitcast(BF16)
                        )
                        del pair_state[k]
                if hT_new is not None:
                    hbf = gpool.tile([128, QW], BF16, tag="hbf", name="hbf")
                    nc.vector.tensor_mul(hbf[:], sig[:, 2 * QW : 3 * QW], tcc[:])
                    # One merged xbar transpose per wave: DMAT cost is ~1.25us
                    # nearly independent of size, so [128, QW] -> [128, nch, 128]
                    # in a single instruction beats per-chunk transposes.
                    nc.sync.dma_start_transpose(
                        hT_new[:, w * nch : (w + 1) * nch, :], hbf[:]
                    )

            def emit_gates(z, w, hT_new, hT8_new, merged=False):
                """Unsplit gate math (bf16 steps): f-first split ACT ordering."""
                sig = gpool.tile([128, 3 * QW], F32, tag="sig", name="sig")
                tg = gpool.tile([128, QW], F32, tag="tg", name="tg")
                if merged:
                    nc.scalar.activation(sig[:], z[:, 0 : 3 * QW], AF.Sigmoid, scale=SCI)
                    nc.scalar.activation(tg[:], z[:, 3 * QW :], AF.Tanh, scale=SCI)
                else:
                    nc.scalar.activation(
                        sig[:, QW : 2 * QW], z[:, QW : 2 * QW], AF.Sigmoid, scale=SCI
                    )
                    nc.scalar.activation(tg[:], z[:, 3 * QW :], AF.Tanh, scale=SCI)
                    nc.scalar.activation(sig[:, 0:QW], z[:, 0:QW], AF.Sigmoid, scale=SCI)
                    nc.scalar.activation(
                        sig[:, 2 * QW : 3 * QW], z[:, 2 * QW : 3 * QW], AF.Sigmoid,
                        scale=SCI,
                    )
                cw = c_sb[:, w * QW : (w + 1) * QW]
                t1 = gpool.tile([128, QW], F32, tag="t1", name="t1")
                t2 = gpool.tile([128, QW], F32, tag="t2", name="t2")
                nc.vector.tensor_mul(t1[:], sig[:, QW : 2 * QW], cw)
                nc.vector.tensor_mul(t2[:], sig[:, 0:QW], tg[:])
                nc.vector.tensor_add(cw, t1[:], t2[:])
                gates_b(sig, w, hT_new, hT8_new)

            # fp8 pair-pack layout: wave w's h lands in pair k = PAIRK[w] at
            # byte slot SLOT[w]; the pair is transposed after its second
            # writer in gate-B order [1, 0, 2, 3].
            PAIRK = {0: 0, 1: 0, 2: 1, 3: 1}
            SLOT = {1: 0, 0: 1, 2: 0, 3: 1}

            def h8chunk(hT8, ch):
                """DoubleRow lhsT AP [128, 2, 128] for chunk ch (units of
                wave ch) out of the pair-transposed container."""
                v = hT8[:, PAIRK[ch], :, :].bitcast(FP8).rearrange(
                    "p a (m s) -> p a m s", s=2
                )
                return v[:, :, :, SLOT[ch]]

            def gates_pair(zP, p, hT_new, hT8_new):
                """Gate math for wave pair p (waves 2p, 2p+1) in ONE set of
                ACT/DVE ops over [128, 2, *] strided APs: halves the ScalarE
                fixed-cost (12 -> 6 ACTs/step), which was delaying the z-PSUM
                reads that gate the next step's x-matmuls."""
                zv = zP[:].rearrange("q (j c) -> q j c", j=2)
                sig = gpool.tile([128, 2, 3 * QW], F32, tag="sigP", name="sigP")
                tg = gpool.tile([128, 2, QW], F32, tag="tgP", name="tgP")
                nc.scalar.activation(sig[:], zv[:, :, 0 : 3 * QW], AF.Sigmoid, scale=SCI)
                nc.scalar.activation(tg[:], zv[:, :, 3 * QW :], AF.Tanh, scale=SCI)
                c3 = c_sb[:, p * 2 * QW : (p + 1) * 2 * QW].rearrange(
                    "q (j c) -> q j c", j=2
                )
                t1 = gpool.tile([128, 2, QW], F32, tag="t1P", name="t1P")
                t2 = gpool.tile([128, 2, QW], F32, tag="t2P", name="t2P")
                nc.vector.tensor_mul(t1[:], sig[:, :, QW : 2 * QW], c3)
                nc.vector.tensor_mul(t2[:], sig[:, :, 0:QW], tg[:])
                nc.vector.tensor_add(c3, t1[:], t2[:])
                tcc = gpool.tile([128, 2, QW], F32, tag="tccP", name="tccP")
                nc.scalar.activation(tcc[:], c3, AF.Tanh)
                if hT8_new is not None:
                    hp8 = gpool.tile([128, QW, 2], FP8, tag="hp8", name="hp8")
                    nc.vector.tensor_mul(
                        hp8[:].rearrange("q m s -> q s m"),
                        sig[:, :, 2 * QW : 3 * QW],
                        tcc[:],
                    )
                    nc.sync.dma_start_transpose(
                        hT8_new[:, p, :, :], hp8[:].bitcast(BF16)
                    )
                if hT_new is not None:
                    hbf = gpool.tile([128, 2, QW], BF16, tag="hbfP", name="hbfP")
                    nc.vector.tensor_mul(hbf[:], sig[:, :, 2 * QW : 3 * QW], tcc[:])
                    for j in range(2):
                        nc.sync.dma_start_transpose(
                            hT_new[:, (2 * p + j) * nch : (2 * p + j + 1) * nch, :],
                            hbf[:, j, :],
                        )

            def emit_step_fp8(x_lhsT, hT8_prev, fp8_out):
                """One fp8 LSTM step (DoubleRow h matmuls).

                PE order: 8 x-matmuls as an h-free prefix; then w0's ch0/ch1,
                w1's full section (closes first), w0's postponed ch2/ch3,
                w2, w3. Closes stay spread for the ScalarE pipeline while
                each chunk's first consumption lands at/after the previous
                step's produce tail (gate chain + pair transpose). Gate
                emission is software-pipelined one wave deep.
                """
                hT_new = (
                    None
                    if fp8_out
                    else hpool.tile([128, NU, 128], BF16, tag="hT", name="hT_new")
                )
                hT8_new = (
                    hpool8.tile([128, 2, 2, 128], BF16, tag="hT8", name="hT8_new")
                    if fp8_out
                    else None
                )
                zs = [
                    zpool.tile([128, WW], F32, tag="z", name=f"z{w}") for w in range(NW)
                ]
                x_only = hT8_prev is None
                # x-prefix in z-slot free order (w1's gates read z first)
                for w in (1, 0, 2, 3):
                    for n in range(NB):
                        nc.tensor.matmul(
                            zs[w][:, n * 512 : (n + 1) * 512],
                            x_lhsT,
                            Wk_sb[:, WW * w + n * 512 : WW * w + (n + 1) * 512],
                            start=True,
                            stop=x_only,
                        )

                A = {}
                def close(w):
                    A[w] = gates_a(zs[w], w)
                def flush(w):
                    gates_b(A.pop(w), w, hT_new, hT8_new)

                if x_only:
                    close(1); close(0); flush(1); flush(0)
                    close(2); close(3); flush(2); flush(3)
                    return hT_new, hT8_new

                def dr(w, ch, stop=False):
                    for n in range(NB):
                        nc.tensor.matmul(
                            zs[w][:, n * 512 : (n + 1) * 512],
                            h8chunk(hT8_prev, ch),
                            Wr8_sb[:, ch, :, WW * w + n * 512 : WW * w + (n + 1) * 512],
                            start=False,
                            stop=stop,
                            perf_mode=DR,
                        )

                dr(0, 0)
                dr(0, 1)
                for ch in range(NCH):          # w1 closes first
                    dr(1, ch, stop=(ch == NCH - 1))
                close(1)
                dr(0, 2)
                dr(0, 3, stop=True)            # w0's postponed chunks
                close(0)
                for ch in range(NCH):
                    dr(2, ch, stop=(ch == NCH - 1))
                # wave 2's z-read (sig/tg) queues BEFORE pair-0's tanh(c)
                # tail so its PSUM bank frees in time for the next step's
                # x-prefix — the dominant measured stall. Pair-0's transpose
                # (feeds ch0/ch1, which have produce slack) slides ~1us.
                close(2); flush(1); flush(0)
                for ch in range(NCH):
                    dr(3, ch, stop=(ch == NCH - 1))
                close(3); flush(2); flush(3)
                return hT_new, hT8_new

            def emit_step(x_lhsT, hT_prev, x_first):
                """One bf16 LSTM step; returns (hT_new bf16, None)."""
                hT_new = hpool.tile([128, NU, 128], BF16, tag="hT", name="hT_new")
                hT8_new = None
                for w in range(NW):
                    base = WW * w
                    z = zpool.tile([128, WW], F32, tag="z", name="z")
                    ks = []
                    if x_first:
                        ks.append(("x", 0))
                    if hT_prev is not None:
                        ks += [("h", u) for u in range(NU)]
                    if not x_first:
                        ks.append(("x", 0))
                    # k-outer / n-inner: the first-emitted matmuls depend on
                    # operands ready earliest (x, then low h chunks), so the
                    # PE can start the next step while the previous step's
                    # late h chunks are still in flight through the
                    # gate-math chain.
                    for ki, (kind, kv) in enumerate(ks):
                        st, sp = ki == 0, ki == len(ks) - 1
                        for n in range(NB):
                            zsl = z[:, n * 512 : (n + 1) * 512]
                            cs = slice(base + n * 512, base + (n + 1) * 512)
                            if kind == "x":
                                nc.tensor.matmul(
                                    zsl, x_lhsT, Wk_sb[:, cs], start=st, stop=sp
                                )
                            else:
                                nc.tensor.matmul(
                                    zsl,
                                    hT_prev[:, kv, :],
                                    Wr_sb[:, kv, cs],
                                    start=st,
                                    stop=sp,
                                )
                    emit_gates(z, w, hT_new, hT8_new, merged=False)
                return hT_new, hT8_new

            def emit_dense(hT_cur, out_idx, feedback, zp=None):
                if zp is None:
                    zp = zpool.tile([128, WW], F32, tag="z", name="zdense")
                pp = zp[0:I, 0:BC]
                for u in range(NU):
                    nc.tensor.matmul(
                        pp,
                        Wd_sb[:, u, :],
                        hT_cur[:, u, :],
                        start=(u == 0),
                        stop=(u == NU - 1),
                    )
                if feedback:
                    pt = gpool.tile([KX, BC], BF16, tag="pT", name="pT")
                    nc.gpsimd.memset(pt[I : I + 1, :], 1.0)
                    nc.scalar.activation(pt[0:I, :], pp, AF.Identity, bias=bd_sb[:])
                else:
                    pt = None
                pf = gpool.tile([I, BC], F32, tag="pf", name="pf")
                nc.scalar.activation(pf[:], pp, AF.Identity, bias=bd_sb[:])
                nc.scalar.dma_start(outD[out_idx], pf[:])
                return pt

            def emit_step_dec(hT_prev, out_idx):
                """Decode step: consumes hT_prev for BOTH the recurrent
                matmuls and the dense head of the PREVIOUS step's output
                (out_idx), whose result pt feeds this step's x-part. The
                dense matmuls run mid-stream (after wave 1's h-section,
                when hT_prev's late chunks have landed) and all x-matmuls
                come after pt, so the PE never idles waiting for the
                h-transpose tail + dense chain between steps."""
                hT_new = hpool.tile([128, NU, 128], BF16, tag="hT", name="hT_new")
                zp = zpool.tile([128, WW], F32, tag="z", name="zdense")
                zs = [
                    zpool.tile([128, WW], F32, tag="z", name=f"z{w}") for w in range(NW)
                ]

                def hsec(w):
                    for u in range(NU):
                        for n in range(NB):
                            nc.tensor.matmul(
                                zs[w][:, n * 512 : (n + 1) * 512],
                                hT_prev[:, u, :],
                                Wr_sb[:, u, WW * w + n * 512 : WW * w + (n + 1) * 512],
                                start=(u == 0),
                                stop=False,
                            )

                def xsec(w):
                    for n in range(NB):
                        nc.tensor.matmul(
                            zs[w][:, n * 512 : (n + 1) * 512],
                            pt[:],
                            Wk_sb[:, WW * w + n * 512 : WW * w + (n + 1) * 512],
                            start=False,
                            stop=True,
                        )

                hsec(0)
                hsec(1)
                pt = emit_dense(hT_prev, out_idx, feedback=True, zp=zp)
                xsec(0)
                emit_gates(zs[0], 0, hT_new, None)
                xsec(1)
                emit_gates(zs[1], 1, hT_new, None)
                for w in (2, 3):
                    hsec(w)
                    xsec(w)
                    emit_gates(zs[w], w, hT_new, None)
                return hT_new

            hT = hT8 = None
            # prefetch input blocks one block (XBLK steps) ahead so the
            # stream DMA never sits on the first x-matmul's critical path
            nblk_used = (n_warm + XBLK - 1) // XBLK
            xtiles = {}
            if nblk_used > 0:
                xtiles[0] = xpool.tile([KX, XBLK * BC], BF16, tag="xblk", name="xblk")
                nc.sync.dma_start(xtiles[0][:], xTbD[0])
            for t in range(n_warm):
                b = t // XBLK
                s = t % XBLK
                f_out = is_fp8(t + 1) if t + 1 < n_steps else False
                xl = xtiles[b][:, s * BC : (s + 1) * BC]
                if is_fp8(t):
                    hT, hT8 = emit_step_fp8(xl, hT8, f_out)
                else:
                    hT, hT8 = emit_step(xl, hT, x_first=True)
                if t % XBLK == 0 and b + 1 < nblk_used:
                    # prefetch the next input block; emitted after the step so
                    # it queues behind this step's critical hT transposes
                    xtiles[b + 1] = xpool.tile([KX, XBLK * BC], BF16, tag="xblk", name="xblk")
                    nc.sync.dma_start(xtiles[b + 1][:], xTbD[b + 1])
                xtiles.pop(b - 1, None)
            if n_dec == 0:
                emit_dense(hT, 0, feedback=False)
            else:
                for s_ in range(1, n_dec + 1):
                    hT = emit_step_dec(hT, s_ - 1)
                emit_dense(hT, n_dec, feedback=False)

    nc.finalize()
    return nc


def prep_in_maps(inputs, Wk, Wr, b, Wd, bd, n_warm=T):
    """Host-side sharding + layout. inputs [B, T, I] fp32; returns 8 in_maps."""
    perm = _gate_perm()
    Wk_aug = np.concatenate(
        [np.asarray(Wk, np.float32), np.asarray(b, np.float32)[None, :]], axis=0
    )
    Wk_p = (Wk_aug[:, perm] * SC).astype(NPBF16)               # [65, 4096] scaled
    Wr_s = np.asarray(Wr, np.float32)[:, perm] * SC            # [1024, 4096] scaled
    Wr_p = Wr_s.reshape(NU, 128, 4 * U).transpose(1, 0, 2).astype(NPBF16).copy()
    # DoubleRow layout: [p, ch, j, n] = Wr_s[ch*256 + j*128 + p, n]
    Wr_8 = (
        Wr_s.reshape(NCH, 2, 128, 4 * U).transpose(2, 0, 1, 3).astype(NPFP8).copy()
    )
    Wd_p = np.asarray(Wd, np.float32).reshape(NU, 128, I).transpose(1, 0, 2).astype(NPBF16).copy()
    bd_c = np.asarray(bd, np.float32).reshape(I, 1).copy()

    x = np.asarray(inputs, np.float32)
    nblk = (n_warm + XBLK - 1) // XBLK
    in_maps = []
    for c in range(NCORES):
        xc = x[c * BC : (c + 1) * BC, :n_warm]                 # [BC, n_warm, I]
        xT = np.transpose(xc, (1, 2, 0))                       # [n_warm, I, BC]
        xTa = np.concatenate([xT, np.ones((n_warm, 1, BC), np.float32)], axis=1)
        if nblk * XBLK != n_warm:
            pad = np.zeros((nblk * XBLK - n_warm, KX, BC), np.float32)
            xTa = np.concatenate([xTa, pad], axis=0)
        xTb = (
            xTa.reshape(nblk, XBLK, KX, BC)
            .transpose(0, 2, 1, 3)
            .reshape(nblk, KX, XBLK * BC)
            .astype(NPBF16)
            .copy()
        )
        in_maps.append(
            {"xTb": xTb, "Wk": Wk_p, "Wr": Wr_p, "Wr8": Wr_8, "Wd": Wd_p, "bdc": bd_c}
        )
    return in_maps


_NC_CACHE = {}


def _get_nc(n_warm, n_dec):
    key = (n_warm, n_dec)
    if key not in _NC_CACHE:
        _NC_CACHE[key] = build_nc(n_warm, n_dec)
    return _NC_CACHE[key]


def run(inputs, Wk, Wr, b, Wd, bd, n_warm, n_dec, trace=False, **kw):
    nc = _get_nc(n_warm, n_dec)
    in_maps = prep_in_maps(inputs, Wk, Wr, b, Wd, bd, n_warm)
    res = run_bass_kernel_spmd(nc, in_maps, list(range(NCORES)), trace=trace, **kw)
    outs = [np.asarray(res.results[c]["out"], np.float32) for c in range(NCORES)]
    # out[c]: [n_dec+1, I, BC] -> preds [B, n_dec+1, I]
    preds = np.concatenate([o.transpose(2, 0, 1) for o in outs], axis=0)
    return preds, res


def kernel(inputs, Wk, Wr, b, Wd, bd, output_indices, output_steps):
    n_dec = int(output_steps) - 1
    preds, _ = run(inputs, Wk, Wr, b, Wd, bd, T, n_dec)
    idx = np.asarray(output_indices, np.int64)
    return np.take(preds, idx, axis=-1).astype(np.float32)
